# revision 1
# baseline (speedup 1.0000x reference)
"""3-layer GCN (message passing) on 8 Trainium2 NeuronCores.

Strategy
--------
Per GCN layer (using linearity: gcn(x) = (A_norm @ x) @ W + b):
  1. agg = A_norm @ h      -- sparse aggregate, done as per-dst-block PE matmuls
                              over dma_gather'ed source rows (fp16 table) with
                              host-built one-hot S matrices (segment sum).
  2. h' = ELU(agg @ W + b) -- dense GEMM per 128-node block + ELU epilogue.
Normalization dinv[src]*dinv[dst] is separable: the gather table holds
dinv-prescaled rows; dst-side dinv is applied at PSUM eviction.

Nodes are sharded contiguously across the 8 cores (6250 each); edges assigned
by destination core. Between layers an AllGather republishes the full fp16
node-feature table to every core's HBM for the next layer's gathers.

dma_gather indices are int16 (< 32768), so the 50000-row table is addressed
via two windows: "lo" = rows [0, 32768) of the full table, "hi" = rows
[17232, 50000) (a sliced AP), with flexible assignment of rows in the overlap
so per-(core, block) chunk counts stay uniform across cores (single SPMD
program).
"""

import numpy as np

N = 50000
E = 800000
F = 128
H = 128
O = 64
NCORES = 8
NLOC = N // NCORES           # 6250
P = 128
NBLK = (NLOC + P - 1) // P   # 49, last block has 106 nodes
LAST_ROWS = NLOC - (NBLK - 1) * P   # 106
HI_BASE = N - 32768          # 17232

_CACHE = {}


def _host_prep(x, edge_index):
    """Build per-core gather indices, S matrices, and scale vectors."""
    src = np.ascontiguousarray(edge_index[0]).astype(np.int64)
    dst = np.ascontiguousarray(edge_index[1]).astype(np.int64)
    loops = np.arange(N, dtype=np.int64)
    src = np.concatenate([src, loops])
    dst = np.concatenate([dst, loops])

    deg = np.bincount(dst, minlength=N).astype(np.float64)  # includes self-loop
    dinv = (1.0 / np.sqrt(deg)).astype(np.float32)

    x_t = (dinv[:, None] * np.asarray(x, dtype=np.float32)).astype(np.float16)

    core = dst // NLOC
    ld = dst - core * NLOC
    blk = ld // P
    col = ld - blk * P
    # src class: 0 = must-lo (src < HI_BASE), 1 = flex, 2 = must-hi
    cls = np.where(src < HI_BASE, 0, np.where(src < 32768, 1, 2)).astype(np.int64)

    key = (core * NBLK + blk) * 4 + cls
    order = np.argsort(key, kind="stable")
    src_s = src[order]
    key_s = key[order]
    col_s = col[order]

    cnt = np.bincount(key_s, minlength=NCORES * NBLK * 4).reshape(NCORES, NBLK, 4)
    n_lo = cnt[:, :, 0]
    n_fx = cnt[:, :, 1]
    n_hi = cnt[:, :, 2]

    # uniform (across cores) chunk counts per block: A lo-chunks + B hi-chunks
    A = np.zeros(NBLK, np.int64)
    B = np.zeros(NBLK, np.int64)
    for b in range(NBLK):
        best = None
        a_min = int(np.max(np.ceil(n_lo[:, b] / P)))
        for a in range(a_min, a_min + 3):
            spill = np.maximum(0, n_fx[:, b] - (P * a - n_lo[:, b]))
            bb = int(np.max(np.ceil((n_hi[:, b] + spill) / P)))
            if best is None or a + bb < best[0] + best[1]:
                best = (a, bb)
        A[b], B[b] = best
    C = A + B
    CT = int(C.sum())
    chunk_base = np.concatenate([[0], np.cumsum(C)]).astype(np.int64)
    slot_base = chunk_base * P

    # per-edge slot assignment
    grp = key_s >> 2          # core*NBLK + blk, sorted
    grp_cnt = np.bincount(grp, minlength=NCORES * NBLK)
    grp_start = np.concatenate([[0], np.cumsum(grp_cnt)])
    rank = np.arange(src_s.shape[0]) - grp_start[grp]
    core_s = grp // NBLK
    blk_s = grp % NBLK
    k_lo = np.minimum(n_lo + n_fx, P * A[None, :])      # [NCORES, NBLK]
    k_lo_e = k_lo[core_s, blk_s]
    is_lo = rank < k_lo_e
    slot_in_blk = np.where(is_lo, rank, P * A[blk_s] + (rank - k_lo_e))
    slot = slot_base[blk_s] + slot_in_blk
    idx_val = np.where(is_lo, src_s, src_s - HI_BASE).astype(np.int64)
    assert idx_val.min() >= 0 and idx_val.max() < 32768
    # sanity: hi slots only for idx that are valid in the hi window
    assert np.all(slot_in_blk < P * C[blk_s])

    idx_arrs = []
    S_arrs = []
    for c in range(NCORES):
        m = core_s == c
        sl = slot[m]
        ia = np.zeros(CT * P, np.int16)
        ia[sl] = idx_val[m].astype(np.int16)
        cols = CT * P // 16
        w = np.zeros((cols, 16), np.int16)
        w.reshape(-1)[:] = ia
        idx_arrs.append(np.tile(w.T.copy(), (8, 1)))
        S = np.zeros((CT, P, P), np.float16)
        S[sl // P, sl % P, col_s[m]] = 1.0
        S_arrs.append(np.ascontiguousarray(S.transpose(1, 0, 2)).reshape(P, CT * P))

    dinv_bcast = []
    dinv_blk = []
    for c in range(NCORES):
        loc = np.zeros(NBLK * P, np.float32)
        loc[:NLOC] = dinv[c * NLOC:(c + 1) * NLOC]
        dinv_bcast.append(np.ascontiguousarray(np.broadcast_to(loc[None, :], (P, NBLK * P))))
        dinv_blk.append(np.ascontiguousarray(loc.reshape(NBLK, P).T))

    meta = dict(A=A.tolist(), B=B.tolist(), C=C.tolist(), CT=CT,
                chunk_base=chunk_base.tolist())
    return x_t, idx_arrs, S_arrs, dinv_bcast, dinv_blk, meta


def _build_program(meta):
    import os
    import concourse.mybir as mybir
    import concourse.tile as tile
    from concourse import bacc

    DBG_LAYERS = int(os.environ.get("GCN_LAYERS", "3"))
    DBG_AG = int(os.environ.get("GCN_AG", "1"))
    DBG_BLOCKS = int(os.environ.get("GCN_BLOCKS", str(NBLK)))
    DBG_REPEAT = int(os.environ.get("GCN_REPEAT", "1"))
    DBG_LINGATHER = int(os.environ.get("GCN_LINGATHER", "0"))
    DBG_GATHERONLY = int(os.environ.get("GCN_GATHERONLY", "0"))

    A, B, C = meta["A"], meta["B"], meta["C"]
    CT = meta["CT"]
    chunk_base = meta["chunk_base"]
    dt = mybir.dt
    ALU = mybir.AluOpType
    ACTF = mybir.ActivationFunctionType

    nc = bacc.Bacc("TRN2", target_bir_lowering=False, num_devices=NCORES)

    t_xt = nc.dram_tensor("x_t", [N, F], dt.float16, kind="ExternalInput")
    t_idx = nc.dram_tensor("idx", [P, CT * 8], dt.int16, kind="ExternalInput")
    t_S = nc.dram_tensor("S", [P, CT * P], dt.float16, kind="ExternalInput")
    t_dbc = nc.dram_tensor("dinv_bcast", [P, NBLK * P], dt.float32, kind="ExternalInput")
    t_dbk = nc.dram_tensor("dinv_blk", [P, NBLK], dt.float32, kind="ExternalInput")
    t_W = [nc.dram_tensor("W1", [F, H], dt.float32, kind="ExternalInput"),
           nc.dram_tensor("W2", [H, H], dt.float32, kind="ExternalInput"),
           nc.dram_tensor("W3", [H, O], dt.float32, kind="ExternalInput")]
    t_b = [nc.dram_tensor("b1_bc", [P, H], dt.float32, kind="ExternalInput"),
           nc.dram_tensor("b2_bc", [P, H], dt.float32, kind="ExternalInput"),
           nc.dram_tensor("b3_bc", [P, O], dt.float32, kind="ExternalInput")]
    t_out = nc.dram_tensor("out", [NLOC, O], dt.float32, kind="ExternalOutput")

    with tile.TileContext(nc) as tc:
        with (
            tc.tile_pool(name="const", bufs=1) as cpool,
            tc.tile_pool(name="gth", bufs=3) as gpool,
            tc.tile_pool(name="smat", bufs=3) as spool,
            tc.tile_pool(name="work", bufs=3) as wpool,
            tc.tile_pool(name="hout", bufs=3) as hpool,
            tc.tile_pool(name="psA", bufs=2, space="PSUM") as psA,
            tc.tile_pool(name="psH", bufs=2, space="PSUM") as psH,
            tc.tile_pool(name="dram", bufs=1, space="DRAM") as dpool,
        ):
            # constants
            idx_t = cpool.tile([P, CT * 8], dt.int16, tag="idx")
            nc.sync.dma_start(idx_t[:], t_idx[:, :])
            dbc_t = cpool.tile([P, NBLK * P], dt.float32, tag="dbc")
            nc.sync.dma_start(dbc_t[:], t_dbc[:, :])
            dbk_t = cpool.tile([P, NBLK], dt.float32, tag="dbk")
            nc.sync.dma_start(dbk_t[:], t_dbk[:, :])
            W_t = []
            b_t = []
            for l in range(3):
                wt = cpool.tile([128, t_W[l].shape[1]], dt.float32, tag=f"W{l}")
                nc.sync.dma_start(wt[:], t_W[l][:, :])
                W_t.append(wt)
                bt = cpool.tile([P, t_b[l].shape[1]], dt.float32, tag=f"b{l}")
                nc.sync.dma_start(bt[:], t_b[l][:, :])
                b_t.append(bt)

            # inter-layer tables (internal DRAM)
            cc_in = [dpool.tile([NLOC, H], dt.float16, tag=f"ccin{l}", name=f"ccin{l}")
                     for l in range(2)]
            cc_out = [dpool.tile([N, H], dt.float16, tag=f"ccout{l}", name=f"ccout{l}")
                      for l in range(2)]

            for rep in range(DBG_REPEAT):
              for l in range(DBG_LAYERS):
                if l == 0:
                    tab_lo = t_xt[:, :]
                    tab_hi = t_xt[HI_BASE:N, :]
                else:
                    tab_lo = cc_out[l - 1][:, :]
                    tab_hi = cc_out[l - 1][HI_BASE:N, :]
                Hout = H if l < 2 else O

                for b in range(DBG_BLOCKS):
                    Cb, Ab = C[b], A[b]
                    cb0 = chunk_base[b]
                    g2 = gpool.tile([P, Cb * P], dt.float16, tag="g")
                    g3 = g2[:, :].rearrange("p (c d) -> p c d", d=P)
                    if DBG_LINGATHER:
                        nc.sync.dma_start(g2[:], t_S[:, cb0 * P:(cb0 + Cb) * P])
                    else:
                        nc.gpsimd.dma_gather(
                            out_ap=g3[:, 0:Ab, :],
                            in_ap=tab_lo,
                            idxs_ap=idx_t[:, cb0 * 8:(cb0 + Ab) * 8],
                            num_idxs=Ab * P,
                            num_idxs_reg=Ab * P,
                            elem_size=P,
                            single_packet=False,
                        )
                        if Cb > Ab:
                            nc.gpsimd.dma_gather(
                                out_ap=g3[:, Ab:Cb, :],
                                in_ap=tab_hi,
                                idxs_ap=idx_t[:, (cb0 + Ab) * 8:(cb0 + Cb) * 8],
                                num_idxs=(Cb - Ab) * P,
                                num_idxs_reg=(Cb - Ab) * P,
                                elem_size=P,
                                single_packet=False,
                            )
                    if DBG_GATHERONLY:
                        continue
                    S_t = spool.tile([P, Cb * P], dt.float16, tag="S")
                    nc.sync.dma_start(S_t[:], t_S[:, cb0 * P:(cb0 + Cb) * P])

                    agg_ps = psA.tile([P, P], dt.float32, tag="aggps")
                    for c in range(Cb):
                        nc.tensor.matmul(
                            out=agg_ps[:, :],
                            lhsT=g2[:, c * P:(c + 1) * P],
                            rhs=S_t[:, c * P:(c + 1) * P],
                            start=(c == 0),
                            stop=(c == Cb - 1),
                        )
                    # dst-side dinv scale at eviction (single PSUM reader)
                    agg = wpool.tile([P, P], dt.float32, tag="agg")
                    nc.vector.tensor_tensor(
                        out=agg[:], in0=agg_ps[:, :],
                        in1=dbc_t[:, b * P:(b + 1) * P], op=ALU.mult)
                    h_ps = psH.tile([P, Hout], dt.float32, tag="hps")
                    nc.tensor.matmul(out=h_ps[:, :], lhsT=agg[:], rhs=W_t[l][:, :],
                                     start=True, stop=True)
                    # epilogue
                    rows = P if b < NBLK - 1 else LAST_ROWS
                    t = wpool.tile([P, Hout], dt.float32, tag="t")
                    nc.vector.tensor_tensor(out=t[:], in0=h_ps[:, :], in1=b_t[l][:, :],
                                            op=ALU.add)
                    if l < 2:
                        m = wpool.tile([P, Hout], dt.float32, tag="m")
                        nc.vector.tensor_scalar(out=m[:], in0=t[:], scalar1=0.0,
                                                scalar2=None, op0=ALU.min)
                        e = wpool.tile([P, Hout], dt.float32, tag="e")
                        nc.scalar.activation(out=e[:], in_=m[:], func=ACTF.Exp)
                        r = wpool.tile([P, Hout], dt.float32, tag="r")
                        nc.vector.tensor_scalar(out=r[:], in0=t[:], scalar1=0.0,
                                                scalar2=-1.0, op0=ALU.max, op1=ALU.add)
                        s = wpool.tile([P, Hout], dt.float32, tag="s")
                        nc.vector.tensor_tensor(out=s[:], in0=r[:], in1=e[:], op=ALU.add)
                        ht = hpool.tile([P, Hout], dt.float16, tag="ht")
                        nc.vector.tensor_scalar(out=ht[:], in0=s[:],
                                                scalar1=dbk_t[:, b:b + 1], scalar2=None,
                                                op0=ALU.mult)
                        nc.sync.dma_start(cc_in[l][b * P:b * P + rows, :], ht[:rows, :])
                    else:
                        nc.sync.dma_start(t_out[b * P:b * P + rows, :], t[:rows, :])

                if l < 2 and l < DBG_LAYERS - 1 and DBG_AG and not DBG_GATHERONLY:
                    nc.gpsimd.collective_compute(
                        "AllGather",
                        mybir.AluOpType.bypass,
                        replica_groups=[list(range(NCORES))],
                        ins=[cc_in[l][:, :].opt()],
                        outs=[cc_out[l][:, :].opt()],
                    )
    nc.compile()
    return nc


def kernel(x, edge_index, W1, b1, W2, b2, W3, b3):
    from concourse.bass_utils import run_bass_kernel_spmd

    x = np.asarray(x)
    edge_index = np.asarray(edge_index)
    x_t, idx_arrs, S_arrs, dinv_bcast, dinv_blk, meta = _host_prep(x, edge_index)

    key = ("prog", meta["CT"], tuple(meta["C"]), tuple(meta["A"]))
    if key not in _CACHE:
        _CACHE[key] = _build_program(meta)
    nc = _CACHE[key]

    b1_bc = np.ascontiguousarray(np.broadcast_to(np.asarray(b1, np.float32)[None, :], (P, H)))
    b2_bc = np.ascontiguousarray(np.broadcast_to(np.asarray(b2, np.float32)[None, :], (P, H)))
    b3_bc = np.ascontiguousarray(np.broadcast_to(np.asarray(b3, np.float32)[None, :], (P, O)))
    W1 = np.ascontiguousarray(W1, np.float32)
    W2 = np.ascontiguousarray(W2, np.float32)
    W3 = np.ascontiguousarray(W3, np.float32)

    in_maps = []
    for c in range(NCORES):
        in_maps.append({
            "x_t": x_t,
            "idx": idx_arrs[c],
            "S": S_arrs[c],
            "dinv_bcast": dinv_bcast[c],
            "dinv_blk": dinv_blk[c],
            "W1": W1, "W2": W2, "W3": W3,
            "b1_bc": b1_bc, "b2_bc": b2_bc, "b3_bc": b3_bc,
        })
    res = run_bass_kernel_spmd(nc, in_maps, core_ids=list(range(NCORES)))
    out = np.concatenate([res.results[c]["out"] for c in range(NCORES)], axis=0)
    return out.astype(np.float32)



# revision 2
# speedup vs baseline: 2.2143x; 2.2143x over previous
"""3-layer GCN (message passing) on 8 Trainium2 NeuronCores — v4.

v3 + continuous cross-block slot packing:
  * Edges of a (5-block group, table part) run are packed contiguously
    across block boundaries — chunk count per run is ceil(total/128) (max
    over cores) instead of per-(block,part) ceils: ~7% fewer gather
    descriptors (padding slots gather row 0; idx=-1 descriptor dropping
    hangs the hardware, so it is not used).
  * A block's aggregation matmuls cover its chunk SPAN (may share boundary
    chunks with neighbors); S one-hot entries disambiguate via column
    offset 128*pos_in_group so foreign edges contribute zero.
  * Everything else as v3: split tables A/B (<32768 rows, pure int16
    addressing), split AllGather overlapped with compute, 4 SWDGE queues,
    on-chip S expansion, dst-side deg^-1/2 at eviction.
"""

import os
import numpy as np

N = 50000
E = 800000
F = 128
H = 128
O = 64
NCORES = 8
NLOC = N // NCORES            # 6250
P = 128
NBLK = (NLOC + P - 1) // P    # 49
NLOCP = NBLK * P              # 6272
GSZ = 5
NGRP = (NBLK + GSZ - 1) // GSZ          # 10 groups
PART_NBLK = [25, 24]
PART_START = [0, 25]
PR = [PART_NBLK[0] * P, PART_NBLK[1] * P]     # 3200, 3072
TBL = [NCORES * PR[0], NCORES * PR[1]]        # 25600, 24576

_CACHE = {}


def _host_prep(x, edge_index):
    src = np.ascontiguousarray(edge_index[0]).astype(np.int64)
    dst = np.ascontiguousarray(edge_index[1]).astype(np.int64)
    loops = np.arange(N, dtype=np.int64)
    src = np.concatenate([src, loops])
    dst = np.concatenate([dst, loops])

    deg = np.bincount(dst, minlength=N).astype(np.float64)
    dinv = (1.0 / np.sqrt(deg)).astype(np.float32)

    xs = (dinv[:, None] * np.asarray(x, dtype=np.float32)).astype(np.float16)
    x_ts = []
    for s in range(2):
        t = np.zeros((TBL[s], F), np.float16)
        for c in range(NCORES):
            lo = PART_START[s] * P
            n_rows = min(NLOC - lo, PR[s])
            t[c * PR[s]:c * PR[s] + n_rows] = xs[c * NLOC + lo:c * NLOC + lo + n_rows]
        x_ts.append(t)

    src_core = src // NLOC
    iloc = src - src_core * NLOC
    bs = iloc // P
    pos = iloc - bs * P
    sidx = np.where(bs < PART_START[1], 0, 1).astype(np.int64)
    row = src_core * np.array(PR)[sidx] + (bs - np.array(PART_START)[sidx]) * P + pos
    assert row.max() < 32768

    core = dst // NLOC
    ld = dst - core * NLOC
    blk = ld // P
    col = ld - blk * P
    grp_of = blk // GSZ
    pig = blk - grp_of * GSZ              # position in group

    # sort by (core, group, part, block)
    key = ((core * NGRP + grp_of) * 2 + sidx) * GSZ + pig
    order = np.argsort(key, kind="stable")
    row_s = row[order]
    key_s = key[order]
    col_s = col[order]
    pig_s = pig[order]

    # counts per (core, group, part, block-in-group)
    cnt4 = np.bincount(key_s, minlength=NCORES * NGRP * 2 * GSZ).reshape(
        NCORES, NGRP, 2, GSZ)
    run_tot = cnt4.sum(axis=3)                        # [NCORES, NGRP, 2]
    run_nch = np.max(np.ceil(run_tot / P).astype(np.int64), axis=0)  # [NGRP, 2]
    CT = int(run_nch.sum())

    run_base = np.zeros((NGRP, 2), np.int64)
    grp_meta = []
    cid = 0
    for g in range(NGRP):
        gm = []
        for s in range(2):
            run_base[g, s] = cid
            gm.append((int(cid), int(run_nch[g, s])))
            cid += int(run_nch[g, s])
        grp_meta.append(gm)
    assert cid == CT

    # per-edge slot: rank within (core, group, part) run
    runkey = key_s // GSZ                              # (core*NGRP+g)*2+s
    rk_cnt = np.bincount(runkey, minlength=NCORES * NGRP * 2)
    rk_start = np.concatenate([[0], np.cumsum(rk_cnt)])
    rank = np.arange(row_s.shape[0]) - rk_start[runkey]
    g_e = (runkey // 2) % NGRP
    s_e = runkey % 2
    core_e = runkey // (2 * NGRP)
    slot = run_base[g_e, s_e] * P + rank

    # block chunk spans (uniform across cores)
    off_pre = np.cumsum(cnt4, axis=3) - cnt4           # start offset per block
    off_post = np.cumsum(cnt4, axis=3)
    span_lo = np.min(off_pre // P, axis=0)             # [NGRP, 2, GSZ]
    span_hi = np.max(np.ceil(off_post / P).astype(np.int64), axis=0)
    span_hi = np.minimum(span_hi, run_nch[:, :, None])

    idx_arrs = []
    scol_arrs = []
    for c in range(NCORES):
        m = core_e == c
        sl = slot[m]
        ia = np.zeros(CT * P, np.int16)   # pads gather row 0 (desc-drop via -1 hangs HW)
        ia[sl] = row_s[m].astype(np.int16)
        w = np.zeros((CT * P // 16, 16), np.int16)
        w.reshape(-1)[:] = ia
        idx_arrs.append(np.tile(w.T.copy(), (8, 1)))
        scm = np.full((P, CT), 9999.0, np.float32)
        scm[sl % P, sl // P] = (col_s[m] + P * pig_s[m]).astype(np.float32)
        scol_arrs.append(np.ascontiguousarray(scm))

    dinv_bcast = []
    dinv_blk = []
    for c in range(NCORES):
        loc = np.zeros(NLOCP, np.float32)
        loc[:NLOC] = dinv[c * NLOC:(c + 1) * NLOC]
        dinv_bcast.append(np.ascontiguousarray(np.broadcast_to(loc[None, :], (P, NLOCP))))
        dinv_blk.append(np.ascontiguousarray(loc.reshape(NBLK, P).T))

    iota = np.ascontiguousarray(
        np.broadcast_to(np.arange(GSZ * P, dtype=np.float16)[None, :], (P, GSZ * P)))

    groups = [list(range(g * GSZ, min((g + 1) * GSZ, NBLK))) for g in range(NGRP)]
    # per block: list of (global_chunk, k=pos_in_group, part)
    blk_chunks = []
    for b in range(NBLK):
        g = b // GSZ
        k = b % GSZ
        lst = []
        for s in range(2):
            for ch in range(int(span_lo[g, s, k]), int(span_hi[g, s, k])):
                lst.append((int(run_base[g, s] + ch), k, s))
        blk_chunks.append(lst)

    meta = dict(CT=CT, groups=groups, grp_meta=grp_meta, blk_chunks=blk_chunks)
    return x_ts, idx_arrs, scol_arrs, dinv_bcast, dinv_blk, iota, meta


def make_in_maps(data):
    x_ts, idx_arrs, scol_arrs, dbc, dbk, iota, meta = _host_prep(
        np.asarray(data["x"]), np.asarray(data["edge_index"]))
    b1 = np.ascontiguousarray(np.broadcast_to(np.asarray(data["b1"], np.float32)[None, :], (P, H)))
    b2 = np.ascontiguousarray(np.broadcast_to(np.asarray(data["b2"], np.float32)[None, :], (P, H)))
    b3 = np.ascontiguousarray(np.broadcast_to(np.asarray(data["b3"], np.float32)[None, :], (P, O)))
    in_maps = []
    for c in range(NCORES):
        in_maps.append({
            "x_tA": x_ts[0], "x_tB": x_ts[1],
            "idx": idx_arrs[c], "scol": scol_arrs[c],
            "dinv_bcast": dbc[c], "dinv_blk": dbk[c], "iota": iota,
            "W1": np.ascontiguousarray(data["W1"], np.float32),
            "W2": np.ascontiguousarray(data["W2"], np.float32),
            "W3": np.ascontiguousarray(data["W3"], np.float32),
            "b1_bc": b1, "b2_bc": b2, "b3_bc": b3,
        })
    return in_maps, meta


def _build_program(meta):
    import concourse.mybir as mybir
    import concourse.tile as tile
    from concourse import bacc

    DBG_LAYERS = int(os.environ.get("GCN_LAYERS", "3"))
    DBG_AG = int(os.environ.get("GCN_AG", "1"))
    DBG_REPEAT = int(os.environ.get("GCN_REPEAT", "1"))
    DBG_GATHERONLY = int(os.environ.get("GCN_GATHERONLY", "0"))
    SHARED_AG = int(os.environ.get("GCN_SHARED_AG", "1"))
    NQUEUES = int(os.environ.get("GCN_QUEUES", "4"))

    CT = meta["CT"]
    groups = meta["groups"]
    grp_meta = meta["grp_meta"]
    blk_chunks = meta["blk_chunks"]
    CMAX = max(len(lst) for lst in blk_chunks)
    NCHMAX = max(gm[0][1] + gm[1][1] for gm in grp_meta)
    dt = mybir.dt
    ALU = mybir.AluOpType
    ACTF = mybir.ActivationFunctionType

    nc = bacc.Bacc("TRN2", target_bir_lowering=False, num_devices=NCORES,
                   num_swdge_queues=NQUEUES)

    t_xt = [nc.dram_tensor("x_tA", [TBL[0], F], dt.float16, kind="ExternalInput"),
            nc.dram_tensor("x_tB", [TBL[1], F], dt.float16, kind="ExternalInput")]
    t_idx = nc.dram_tensor("idx", [P, CT * 8], dt.int16, kind="ExternalInput")
    t_scol = nc.dram_tensor("scol", [P, CT], dt.float32, kind="ExternalInput")
    t_dbc = nc.dram_tensor("dinv_bcast", [P, NLOCP], dt.float32, kind="ExternalInput")
    t_dbk = nc.dram_tensor("dinv_blk", [P, NBLK], dt.float32, kind="ExternalInput")
    t_iota = nc.dram_tensor("iota", [P, GSZ * P], dt.float16, kind="ExternalInput")
    t_W = [nc.dram_tensor("W1", [F, H], dt.float32, kind="ExternalInput"),
           nc.dram_tensor("W2", [H, H], dt.float32, kind="ExternalInput"),
           nc.dram_tensor("W3", [H, O], dt.float32, kind="ExternalInput")]
    t_b = [nc.dram_tensor("b1_bc", [P, H], dt.float32, kind="ExternalInput"),
           nc.dram_tensor("b2_bc", [P, H], dt.float32, kind="ExternalInput"),
           nc.dram_tensor("b3_bc", [P, O], dt.float32, kind="ExternalInput")]
    t_out = nc.dram_tensor("out", [NLOCP, O], dt.float32, kind="ExternalOutput")

    with tile.TileContext(nc) as tc:
        with (
            tc.tile_pool(name="const", bufs=1) as cpool,
            tc.tile_pool(name="gth", bufs=2) as gpool,
            tc.tile_pool(name="smat", bufs=3) as spool,
            tc.tile_pool(name="work", bufs=3) as wpool,
            tc.tile_pool(name="hout", bufs=2) as hpool,
            tc.tile_pool(name="psA", bufs=4, space="PSUM") as psA,
            tc.tile_pool(name="psH", bufs=4, space="PSUM") as psH,
            tc.tile_pool(name="dram", bufs=1, space="DRAM") as dpool,
        ):
            idx_t = cpool.tile([P, CT * 8], dt.int16, tag="idx")
            nc.sync.dma_start(idx_t[:], t_idx[:, :])
            scol_t = cpool.tile([P, CT], dt.float32, tag="scol")
            nc.sync.dma_start(scol_t[:], t_scol[:, :])
            dbc_t = cpool.tile([P, NLOCP], dt.float32, tag="dbc")
            nc.sync.dma_start(dbc_t[:], t_dbc[:, :])
            dbk_t = cpool.tile([P, NBLK], dt.float32, tag="dbk")
            nc.sync.dma_start(dbk_t[:], t_dbk[:, :])
            iota_t = cpool.tile([P, GSZ * P], dt.float16, tag="iota")
            nc.sync.dma_start(iota_t[:], t_iota[:, :])
            W_t, b_t = [], []
            for l in range(3):
                wt = cpool.tile([128, t_W[l].shape[1]], dt.float32, tag=f"W{l}")
                nc.sync.dma_start(wt[:], t_W[l][:, :])
                W_t.append(wt)
                bt = cpool.tile([P, t_b[l].shape[1]], dt.float32, tag=f"b{l}")
                nc.sync.dma_start(bt[:], t_b[l][:, :])
                b_t.append(bt)

            cc_in = [[dpool.tile([PR[s], H], dt.float16, tag=f"ccin{l}{s}",
                                 name=f"ccin{l}{s}") for s in range(2)]
                     for l in range(2)]
            ag_kw = dict(addr_space="Shared") if SHARED_AG else {}

            qcount = [0]

            def next_q():
                q = qcount[0] % NQUEUES
                qcount[0] += 1
                return q

            # zero-init gather buffers: chunks dropped by trailing -1 idx
            # leave stale SBUF contents; S zeros them, but they must be finite
            for zi in range(2):
                gb = gpool.tile([P, NCHMAX * P], dt.float16, tag="g",
                                name=f"gz{zi}")
                nc.vector.memset(gb[:], 0.0)

            for rep in range(DBG_REPEAT):
              cc_out = [[dpool.tile([TBL[s], H], dt.float16,
                                    tag=f"ccout{l}{s}r{rep}",
                                    name=f"ccout{l}{s}r{rep}", **ag_kw)
                         for s in range(2)]
                        for l in range(2)]
              for l in range(DBG_LAYERS):
                tabs = t_xt if l == 0 else cc_out[l - 1]
                is_final = (l == 2)
                Hout = O if is_final else H
                ho = hpool.tile([P, NBLK * Hout],
                                dt.float32 if is_final else dt.float16,
                                tag="hof" if is_final else "ho")

                for gi, blks in enumerate(groups):
                    gm = grp_meta[gi]
                    gt = gpool.tile([P, NCHMAX * P], dt.float16, tag="g")
                    g3 = gt[:, :].rearrange("p (c d) -> p c d", d=P)
                    goff = [0, gm[0][1]]
                    for s in range(2):
                        base, nch = gm[s]
                        if nch == 0:
                            continue
                        nc.gpsimd.dma_gather(
                            out_ap=g3[:, goff[s]:goff[s] + nch, :],
                            in_ap=tabs[s][:, :],
                            idxs_ap=idx_t[:, base * 8:(base + nch) * 8],
                            num_idxs=nch * P,
                            num_idxs_reg=nch * P,
                            elem_size=P,
                            single_packet=False,
                            queue_num=next_q(),
                        )
                    if DBG_GATHERONLY:
                        continue

                    for b in blks:
                        chunks = blk_chunks[b]
                        Cb = len(chunks)
                        S_t = spool.tile([P, CMAX * P], dt.float16, tag="S")
                        for i, (c, k, s) in enumerate(chunks):
                            nc.vector.tensor_scalar(
                                out=S_t[:, i * P:(i + 1) * P],
                                in0=iota_t[:, k * P:(k + 1) * P],
                                scalar1=scol_t[:, c:c + 1], scalar2=None,
                                op0=ALU.is_equal)
                        agg_ps = psA.tile([P, P], dt.float32, tag="aggps")
                        for i, (c, k, s) in enumerate(chunks):
                            pos = goff[s] + (c - gm[s][0])
                            nc.tensor.matmul(
                                out=agg_ps[:, :],
                                lhsT=gt[:, pos * P:(pos + 1) * P],
                                rhs=S_t[:, i * P:(i + 1) * P],
                                start=(i == 0),
                                stop=(i == Cb - 1),
                            )
                        agg = wpool.tile([P, P], dt.float32, tag="agg")
                        nc.vector.tensor_tensor(
                            out=agg[:], in0=agg_ps[:, :],
                            in1=dbc_t[:, b * P:(b + 1) * P], op=ALU.mult)
                        h_ps = psH.tile([P, Hout], dt.float32, tag="hps")
                        nc.tensor.matmul(out=h_ps[:, :], lhsT=agg[:], rhs=W_t[l][:, :],
                                         start=True, stop=True)
                        if not is_final:
                            t = wpool.tile([P, Hout], dt.float32, tag="t")
                            nc.vector.tensor_tensor(out=t[:], in0=h_ps[:, :],
                                                    in1=b_t[l][:, :], op=ALU.add)
                            m = wpool.tile([P, Hout], dt.float32, tag="m")
                            nc.vector.tensor_scalar(out=m[:], in0=t[:], scalar1=0.0,
                                                    scalar2=None, op0=ALU.min)
                            e = wpool.tile([P, Hout], dt.float32, tag="e")
                            nc.scalar.activation(out=e[:], in_=m[:], func=ACTF.Exp)
                            r = wpool.tile([P, Hout], dt.float32, tag="r")
                            nc.vector.tensor_scalar(out=r[:], in0=t[:], scalar1=0.0,
                                                    scalar2=-1.0, op0=ALU.max,
                                                    op1=ALU.add)
                            s_ = wpool.tile([P, Hout], dt.float32, tag="s")
                            nc.vector.tensor_tensor(out=s_[:], in0=r[:], in1=e[:],
                                                    op=ALU.add)
                            nc.vector.tensor_scalar(
                                out=ho[:, b * Hout:(b + 1) * Hout], in0=s_[:],
                                scalar1=dbk_t[:, b:b + 1], scalar2=None,
                                op0=ALU.mult)
                        else:
                            nc.vector.tensor_tensor(
                                out=ho[:, b * Hout:(b + 1) * Hout],
                                in0=h_ps[:, :], in1=b_t[l][:, :], op=ALU.add)

                    if (not DBG_GATHERONLY and not is_final and l < 2
                            and blks[-1] == PART_START[1] - 1):
                        hoA = ho[:, 0:PART_NBLK[0] * Hout].rearrange(
                            "p (b j) -> p b j", j=Hout)
                        dstA = cc_in[l][0][:, :].rearrange("(b p) j -> p b j", p=P)
                        nc.sync.dma_start(dstA, hoA)
                        if DBG_AG:
                            nc.gpsimd.collective_compute(
                                "AllGather", mybir.AluOpType.bypass,
                                replica_groups=[list(range(NCORES))],
                                ins=[cc_in[l][0][:, :].opt()],
                                outs=[cc_out[l][0][:, :].opt()],
                            )

                if DBG_GATHERONLY:
                    continue
                if is_final:
                    ho3 = ho[:, :].rearrange("p (b j) -> p b j", j=Hout)
                    dst3 = t_out[:, :].rearrange("(b p) j -> p b j", p=P)
                    nc.sync.dma_start(dst3, ho3)
                elif l < 2:
                    hoB = ho[:, PART_NBLK[0] * Hout:].rearrange(
                        "p (b j) -> p b j", j=Hout)
                    dstB = cc_in[l][1][:, :].rearrange("(b p) j -> p b j", p=P)
                    nc.sync.dma_start(dstB, hoB)
                    if DBG_AG:
                        nc.gpsimd.collective_compute(
                            "AllGather", mybir.AluOpType.bypass,
                            replica_groups=[list(range(NCORES))],
                            ins=[cc_in[l][1][:, :].opt()],
                            outs=[cc_out[l][1][:, :].opt()],
                        )
    nc.compile()
    return nc


def kernel(x, edge_index, W1, b1, W2, b2, W3, b3):
    from concourse.bass_utils import run_bass_kernel_spmd

    data = {"x": x, "edge_index": edge_index, "W1": W1, "b1": b1,
            "W2": W2, "b2": b2, "W3": W3, "b3": b3}
    in_maps, meta = make_in_maps(data)

    key = ("prog", meta["CT"], hash(repr(meta)))
    if key not in _CACHE:
        _CACHE[key] = _build_program(meta)
    nc = _CACHE[key]

    res = run_bass_kernel_spmd(nc, in_maps, core_ids=list(range(NCORES)))
    out = np.concatenate([res.results[c]["out"][:NLOC] for c in range(NCORES)], axis=0)
    return out.astype(np.float32)
